# revision 1
# baseline (speedup 1.0000x reference)
"""Bass/Trainium2 kernel for nn_Loss_25546465477236 (YOLO-style detection loss).

Contract: kernel(**inputs) takes FULL unsharded inputs
  pred_tensor  [1024, 80, 80, 5] f32
  target_boxes [1024, 80, 80, 4] f32
  obj_mask     [1024, 80, 80]    i32
and returns the FULL scalar loss (f32), matching the jax reference.

Strategy: pure data parallel over 8 NeuronCores (batch 1024 -> 8 x 128).
Per core, the 128 batch items map to the 128 SBUF partitions and the
80*80=6400 cells per item are streamed along the free dimension in chunks.

Math (exact algebraic simplification of the reference):
  Because the reference's (buggy) xyxy conversion uses w/S as the center for
  BOTH axes, the x-overlap reduces exactly to  iw = min(pw, tw).
  The y-overlap (scaled by 2):
      ih2 = relu(2*ph - s),  s = relu(e+d) + relu(e-d) = relu(max(2e, e+|d|))
      with d = (pw-tw)/40, e = ph-th
  inter2 = min(pw,tw)*ih2            (= 2*inter)
  denom2 = 2*(pw*ph + tw*th) - inter2 (= 2*union)
  iou    = inter2 / denom2

  loss_sum = 5*sum(m*(dx^2+dy^2)) + 5*sum(m*(d1^2+d2^2))
           + sum(m*(pc-iou)^2) + 0.5*(sum(pc^2) - sum(m*pc^2))
  loss = loss_sum / 1024
  (dx=px-tx, dy=py-ty, d1=sqrt(pw)-sqrt(tw), d2=sqrt(ph)-sqrt(th), m in {0,1})

All masked quadratic sums use m^2 = m:  sum(m*x^2) = sum((m*x)^2), computed
with one DVE multiply + one ScalarE Square-with-accum per stream.  Per-chunk
partial sums land in per-chunk accumulator slots; the host does the final
(tiny) reduction in float64.
"""

import os
import numpy as np

import concourse.bass as bass
import concourse.bacc as bacc
import concourse.mybir as mybir
import concourse.tile as tile
from concourse.bass_utils import run_bass_kernel_spmd

N_CORES = 8
B = 1024
PB = B // N_CORES          # 128 batch items per core -> partition dim
CELLS = 80 * 80            # 6400 cells per batch item
F = 400                    # cells per chunk (free-dim)
NCHUNK = CELLS // F
NG = 7                     # accumulator groups: mdx2,mdy2,md1sq,md2sq,mpd2,mpc2,pc2

f32 = mybir.dt.float32
i32 = mybir.dt.int32
AL = mybir.AluOpType
AF = mybir.ActivationFunctionType

S_INV40 = 1.0 / 40.0       # 2/S with S=80


def build_nc(F=F):
    nchunk = CELLS // F
    nc = bacc.Bacc("TRN2", target_bir_lowering=False, debug=False,
                   num_devices=N_CORES)

    pred_d = nc.dram_tensor("pred", [PB, CELLS * 5], f32, kind="ExternalInput")
    targ_d = nc.dram_tensor("targ", [PB, CELLS * 4], f32, kind="ExternalInput")
    mask_d = nc.dram_tensor("mask", [PB, CELLS], i32, kind="ExternalInput")
    out_d = nc.dram_tensor("acc", [PB, NG * nchunk], f32, kind="ExternalOutput")

    with tile.TileContext(nc) as tc:
        with (
            tc.tile_pool(name="io", bufs=2) as io,
            tc.tile_pool(name="st", bufs=2) as st,
            tc.tile_pool(name="mm", bufs=6) as mmp,
            tc.tile_pool(name="tr", bufs=2) as trp,
            tc.tile_pool(name="accp", bufs=1) as accp,
        ):
            acc = accp.tile([PB, NG * nchunk], f32, tag="acc")

            for c in range(nchunk):
                pt = io.tile([PB, 5 * F], f32, tag="pt")
                tt = io.tile([PB, 4 * F], f32, tag="tt")
                mt = io.tile([PB, F], i32, tag="mt")
                nc.sync.dma_start(pt[:], pred_d[:, c * 5 * F:(c + 1) * 5 * F])
                nc.sync.dma_start(tt[:], targ_d[:, c * 4 * F:(c + 1) * 4 * F])
                nc.sync.dma_start(mt[:], mask_d[:, c * F:(c + 1) * F])

                ptc = pt[:].rearrange("p (n c) -> p c n", c=5)   # [128,5,F]
                ttc = tt[:].rearrange("p (n c) -> p c n", c=4)   # [128,4,F]
                pw, ph, pcf = ptc[:, 2, :], ptc[:, 3, :], ptc[:, 4, :]
                tw, th = ttc[:, 2, :], ttc[:, 3, :]

                # mask int32 {0,1} -> f32 (ScalarE copy converts dtype)
                mf = st.tile([PB, F], f32, tag="mf")
                nc.scalar.copy(mf[:], mt[:])

                # d4 planes: [dx | dy | dw | e]
                d4 = st.tile([PB, 4 * F], f32, tag="d4")
                d4c = d4[:].rearrange("p (c n) -> p c n", c=4)
                nc.vector.tensor_tensor(d4c, ptc[:, 0:4, :], ttc[:, 0:4, :],
                                        AL.subtract)
                dx, dy = d4[:, 0:F], d4[:, F:2 * F]
                dw, e = d4[:, 2 * F:3 * F], d4[:, 3 * F:4 * F]

                # sqrt planes [sqrt(pw)|sqrt(ph)], [sqrt(tw)|sqrt(th)]
                rp = st.tile([PB, 2 * F], f32, tag="rp")
                nc.scalar.activation(rp[:].rearrange("p (c n) -> p c n", c=2),
                                     ptc[:, 2:4, :], AF.Sqrt)
                rt = st.tile([PB, 2 * F], f32, tag="rt")
                nc.scalar.activation(rt[:].rearrange("p (c n) -> p c n", c=2),
                                     ttc[:, 2:4, :], AF.Sqrt)
                dwh = st.tile([PB, 2 * F], f32, tag="dwh")
                nc.vector.tensor_tensor(dwh[:], rp[:], rt[:], AL.subtract)
                d1, d2 = dwh[:, 0:F], dwh[:, F:2 * F]

                # h = relu(max(e, e/2 + |dw|/80));  ih = relu(ph - h)
                absd = st.tile([PB, F], f32, tag="absd")
                nc.scalar.activation(absd[:], dw, AF.Abs, 0.0, 1.0 / 80.0)
                eh = st.tile([PB, F], f32, tag="eh")
                nc.vector.tensor_scalar_mul(eh[:], e, 0.5)
                t1 = st.tile([PB, F], f32, tag="t1")
                nc.vector.tensor_tensor(t1[:], eh[:], absd[:], AL.add)
                spre = st.tile([PB, F], f32, tag="spre")
                nc.vector.tensor_tensor(spre[:], e, t1[:], AL.max)
                h = st.tile([PB, F], f32, tag="h")
                nc.scalar.activation(h[:], spre[:], AF.Relu)
                ihx = st.tile([PB, F], f32, tag="ihx")
                nc.vector.tensor_tensor(ihx[:], ph, h[:], AL.subtract)
                ih = st.tile([PB, F], f32, tag="ih")
                nc.scalar.activation(ih[:], ihx[:], AF.Relu)

                # iou = inter / union
                wmin = st.tile([PB, F], f32, tag="wmin")
                nc.vector.tensor_tensor(wmin[:], pw, tw, AL.min)
                wp = st.tile([PB, F], f32, tag="wp")
                nc.gpsimd.tensor_tensor(wp[:], pw, ph, AL.mult)
                wt = st.tile([PB, F], f32, tag="wt")
                nc.gpsimd.tensor_tensor(wt[:], tw, th, AL.mult)
                s2 = st.tile([PB, F], f32, tag="s2")
                nc.gpsimd.tensor_tensor(s2[:], wp[:], wt[:], AL.add)
                inter = st.tile([PB, F], f32, tag="inter")
                nc.vector.tensor_tensor(inter[:], wmin[:], ih[:], AL.mult)
                denom = st.tile([PB, F], f32, tag="denom")
                nc.gpsimd.tensor_tensor(denom[:], s2[:], inter[:], AL.subtract)
                r = st.tile([PB, F], f32, tag="r")
                nc.vector.reciprocal_approx_fast(r[:], denom[:])
                niou = st.tile([PB, F], f32, tag="niou")
                nc.gpsimd.tensor_tensor(niou[:], inter[:], r[:], AL.mult)
                pd = st.tile([PB, F], f32, tag="pd")
                nc.vector.tensor_tensor(pd[:], pcf, niou[:], AL.subtract)

                # masked quadratic sums: acc[:, g*nchunk+c] = sum((m*x)^2)
                for g, src in enumerate((dx, dy, d1, d2, pd[:], pcf)):
                    mm = mmp.tile([PB, F], f32, tag="mm")
                    eng = nc.gpsimd if g == 5 else nc.vector
                    eng.tensor_tensor(mm[:], src, mf[:], AL.mult)
                    tr = trp.tile([PB, F], f32, tag="tr")
                    slot = acc[:, g * nchunk + c:g * nchunk + c + 1]
                    nc.scalar.activation(tr[:], mm[:], AF.Square,
                                         accum_out=slot)
                # unmasked sum(pc^2)
                tr = trp.tile([PB, F], f32, tag="tr")
                slot = acc[:, 6 * nchunk + c:6 * nchunk + c + 1]
                nc.scalar.activation(tr[:], pcf, AF.Square, accum_out=slot)

            nc.sync.dma_start(out_d[:], acc[:])

    nc.compile()
    return nc


_nc_cache = {}


def get_nc(F=F):
    if F not in _nc_cache:
        _nc_cache[F] = build_nc(F)
    return _nc_cache[F]


def make_in_maps(pred_tensor, target_boxes, obj_mask):
    pred = np.ascontiguousarray(np.asarray(pred_tensor, dtype=np.float32))
    targ = np.ascontiguousarray(np.asarray(target_boxes, dtype=np.float32))
    mask = np.ascontiguousarray(np.asarray(obj_mask, dtype=np.int32))
    pred = pred.reshape(N_CORES, PB, CELLS * 5)
    targ = targ.reshape(N_CORES, PB, CELLS * 4)
    mask = mask.reshape(N_CORES, PB, CELLS)
    return [
        {"pred": pred[k], "targ": targ[k], "mask": mask[k]}
        for k in range(N_CORES)
    ]


def combine_accs(accs, nchunk=NCHUNK):
    """accs: list/array of per-core [PB, NG*nchunk] f32 partial sums."""
    a = np.asarray(accs, dtype=np.float64)      # [ncores, PB, NG*nchunk]
    a = a.reshape(len(accs), PB, NG, nchunk)
    S = a.sum(axis=(0, 1, 3))                   # [NG]
    s_xy = S[0] + S[1]
    s_wh = S[2] + S[3]
    s_obj = S[4]
    s_mpc2 = S[5]
    s_pc2 = S[6]
    loss_sum = 5.0 * (s_xy + s_wh) + s_obj + 0.5 * (s_pc2 - s_mpc2)
    return np.float32(loss_sum / B)


def kernel(pred_tensor, target_boxes, obj_mask):
    nc = get_nc()
    in_maps = make_in_maps(pred_tensor, target_boxes, obj_mask)
    res = run_bass_kernel_spmd(nc, in_maps, core_ids=list(range(N_CORES)))
    accs = [res.results[k]["acc"] for k in range(N_CORES)]
    return combine_accs(accs)


if __name__ == "__main__":
    rng = np.random.default_rng(0)
    p = rng.random((B, 80, 80, 5), dtype=np.float32)
    t = rng.random((B, 80, 80, 4), dtype=np.float32)
    m = rng.integers(0, 2, size=(B, 80, 80)).astype(np.int32)
    print("loss:", kernel(p, t, m))



# revision 3
# speedup vs baseline: 1.6350x; 1.6350x over previous
"""Bass/Trainium2 kernel for nn_Loss_25546465477236 (YOLO-style detection loss).

Contract: kernel(**inputs) takes FULL unsharded inputs
  pred_tensor  [1024, 80, 80, 5] f32
  target_boxes [1024, 80, 80, 4] f32
  obj_mask     [1024, 80, 80]    i32
and returns the FULL scalar loss (f32), matching the jax reference.

Strategy: pure data parallel over 8 NeuronCores (batch 1024 -> 8 x 128).
Per core, the 128 batch items map to the 128 SBUF partitions and the
80*80=6400 cells per item stream along the free dimension in chunks.

Host marshaling (pure layout, no math): inputs are repacked plane-major
  X [N, 9, 6400] f32 with planes [px,tx,py,ty,pw,tw,ph,th,pc]
  M [N, 6400] u8  (obj_mask 0/1, lossless narrowing)
so every on-chip operand is unit-stride and DMA moves large contiguous
runs per partition.

Math (validated against the reference in f64; bf16 pipeline rel err ~6e-5):
  Because the reference's xyxy conversion uses w/S as the center for BOTH
  axes, x-overlap = min(pw,tw) exactly, and the y-overlap reduces to
      ih = relu(ph - relu((e + max(e, |dw|/40))/2)),  dw=pw-tw, e=ph-th
  inter = min(pw,tw)*ih;  union = pw*ph + tw*th - inter;  iou = inter/union
  (sqrt-loss identity) (sqrt(pw)-sqrt(tw))^2 = pw + tw - 2*sqrt(pw*tw)

  Masking: the wh planes and pc are multiplied by m up front; for m=0 the
  whole iou chain collapses to 0 and union to 0, so denom gets +eps to keep
  1/denom finite -> those cells contribute exactly 0 to every masked sum.

Engine split (all ~85-92us, right at the 32.8MB/358GBps DMA roofline):
  GpSimd: mask the 4 wh planes (f32*f32->bf16), dxy = pxy - txy (->bf16)
  Vector: bf16 tensor_tensor chain at 2x mode; fp32 only for denom/recip
  Scalar: mask converts, abs/relu, and all 6 accumulating reductions
Per-chunk partial sums land in per-(group,chunk) slots; host combines in f64.
"""

import numpy as np

import concourse.bass as bass
import concourse.bacc as bacc
import concourse.mybir as mybir
import concourse.tile as tile
from concourse.bass_utils import run_bass_kernel_spmd

N_CORES = 8
B = 1024
PB = B // N_CORES          # 128 batch items per core -> partition dim
CELLS = 80 * 80            # 6400 cells per batch item
F = 1280                   # cells per chunk (free-dim)
NCHUNK = CELLS // F
NG = 6                     # accum groups: A12,A3,A4,A5,A6,A7

f32 = mybir.dt.float32
bf16 = mybir.dt.bfloat16
u8 = mybir.dt.uint8
AL = mybir.AluOpType
AF = mybir.ActivationFunctionType

EPS = 1e-12


def build_nc(F=F):
    nchunk = CELLS // F
    nc = bacc.Bacc("TRN2", target_bir_lowering=False, debug=False,
                   num_devices=N_CORES)

    x_d = nc.dram_tensor("x", [PB, 9 * CELLS], f32, kind="ExternalInput")
    m_d = nc.dram_tensor("m", [PB, CELLS], u8, kind="ExternalInput")
    out_d = nc.dram_tensor("acc", [PB, NG * nchunk], f32, kind="ExternalOutput")

    x3_d = x_d[:].rearrange("p (n c) -> p n c", n=9)

    with tile.TileContext(nc) as tc:
        with (
            tc.tile_pool(name="io", bufs=2) as io,
            tc.tile_pool(name="pre", bufs=2) as pre,
            tc.tile_pool(name="wk", bufs=1) as wk,
            tc.tile_pool(name="accp", bufs=1) as accp,
        ):
            acc = accp.tile([PB, NG * nchunk], f32, tag="acc")

            for c in range(nchunk):
                xt = io.tile([PB, 9 * F], f32, tag="xt")
                mt = io.tile([PB, F], u8, tag="mt")
                nc.sync.dma_start(
                    xt[:].rearrange("p (n f) -> p n f", n=9),
                    x3_d[:, :, c * F:(c + 1) * F])
                nc.sync.dma_start(mt[:], m_d[:, c * F:(c + 1) * F])

                x3 = xt[:].rearrange("p (n f) -> p n f", n=9)
                # xy block as [p, pair, {p|t}, f]
                xyv = xt[:, 0:4 * F].rearrange("p (n two f) -> p n two f",
                                               n=2, two=2)
                pc_plane = x3[:, 8, :]

                # mask converts (u8 -> f32 for gpsimd, u8 -> bf16 for DVE)
                mf = pre.tile([PB, F], f32, tag="mf")
                nc.scalar.copy(mf[:], mt[:])
                mb = pre.tile([PB, F], bf16, tag="mb")
                nc.scalar.copy(mb[:], mt[:])

                # GpSimd: masked wh planes [mpw|mtw|mph|mth] (f32*f32 -> bf16)
                mwh4 = pre.tile([PB, 4 * F], bf16, tag="mwh4")
                nc.gpsimd.tensor_tensor(
                    mwh4[:].rearrange("p (n f) -> p n f", n=4),
                    x3[:, 4:8, :],
                    mf[:].unsqueeze(1).broadcast_to((PB, 4, F)),
                    AL.mult)
                # GpSimd: dxy = [px-tx | py-ty]  (f32 -> bf16)
                dxy = pre.tile([PB, 2 * F], bf16, tag="dxy")
                nc.gpsimd.tensor_tensor(
                    dxy[:].rearrange("p (n f) -> p n f", n=2),
                    xyv[:, :, 0, :], xyv[:, :, 1, :], AL.subtract)

                whv = mwh4[:].rearrange("p (n two f) -> p n two f", n=2, two=2)
                mpw, mtw = mwh4[:, 0:F], mwh4[:, F:2 * F]
                mph = mwh4[:, 2 * F:3 * F]

                # mpc = pc * m (f32 -> bf16)
                mpc = wk.tile([PB, F], bf16, tag="mpc")
                nc.vector.tensor_tensor(mpc[:], pc_plane, mf[:], AL.mult)

                # u2 = [mpw*mtw | mph*mth]
                u2 = wk.tile([PB, 2 * F], bf16, tag="u2")
                u2v = u2[:].rearrange("p (n f) -> p n f", n=2)
                nc.vector.tensor_tensor(u2v, whv[:, :, 0, :], whv[:, :, 1, :],
                                        AL.mult)
                # dwe = [dw | e]
                dwe = wk.tile([PB, 2 * F], bf16, tag="dwe")
                dwev = dwe[:].rearrange("p (n f) -> p n f", n=2)
                nc.vector.tensor_tensor(dwev, whv[:, :, 0, :], whv[:, :, 1, :],
                                        AL.subtract)
                dw, e = dwe[:, 0:F], dwe[:, F:2 * F]

                # t1 <- absd = |dw|/40   (in place over dw)
                nc.scalar.activation(dw, dw, AF.Abs, 0.0, 1.0 / 40.0)
                # t1 chain in one buffer
                t1 = wk.tile([PB, F], bf16, tag="t1")
                nc.vector.tensor_tensor(t1[:], e, dw, AL.max)       # mx
                nc.vector.tensor_tensor(t1[:], e, t1[:], AL.add)    # s0
                nc.vector.tensor_scalar(t1[:], t1[:], 0.5, 0.0,
                                        AL.mult, AL.max)            # q
                nc.vector.tensor_tensor(t1[:], mph, t1[:], AL.subtract)  # ihx
                nc.scalar.activation(t1[:], t1[:], AF.Relu)         # ih

                # wpwt = [mpw*mph | mtw*mth]
                wpwt = wk.tile([PB, 2 * F], bf16, tag="wpwt")
                nc.vector.tensor_tensor(wpwt[:], mwh4[:, 0:2 * F],
                                        mwh4[:, 2 * F:4 * F], AL.mult)
                # s2 = wp + wt (in place over wp half)
                s2 = wpwt[:, 0:F]
                nc.vector.tensor_tensor(s2, s2, wpwt[:, F:2 * F], AL.add)
                # wmin = min(mpw, mtw)
                wmin = wk.tile([PB, F], bf16, tag="wmin")
                nc.vector.tensor_tensor(wmin[:], mpw, mtw, AL.min)
                # inter = wmin * ih (in place over wmin)
                nc.vector.tensor_tensor(wmin[:], wmin[:], t1[:], AL.mult)
                inter = wmin[:]

                # denom = (s2 + eps) - inter   (fp32 out for recip)
                denom = wk.tile([PB, F], f32, tag="denom")
                nc.vector.scalar_tensor_tensor(denom[:], s2, EPS, inter,
                                               AL.add, AL.subtract)
                r = wk.tile([PB, F], f32, tag="r")
                nc.vector.reciprocal_approx_fast(r[:], denom[:])
                # rbf: f32 -> bf16 (reuse t1: ih is dead after inter)
                nc.vector.tensor_copy(t1[:], r[:])
                # niou = inter * rbf (in place over inter)
                nc.vector.tensor_tensor(inter, inter, t1[:], AL.mult)
                # pd = mpc - niou (in place over niou)
                nc.vector.tensor_tensor(inter, mpc[:], inter, AL.subtract)
                pd = inter

                # mdxy = dxy * m (in place over dxy)
                nc.vector.tensor_tensor(
                    dxy[:].rearrange("p (n f) -> p n f", n=2),
                    dxy[:].rearrange("p (n f) -> p n f", n=2),
                    mb[:].unsqueeze(1).broadcast_to((PB, 2, F)),
                    AL.mult)

                # ---- accumulating reductions (ScalarE), outputs in place ----
                def slot(g):
                    return acc[:, g * nchunk + c:g * nchunk + c + 1]

                nc.scalar.activation(dxy[:], dxy[:], AF.Square,
                                     accum_out=slot(0))             # A12
                nc.scalar.activation(mwh4[:], mwh4[:], AF.Copy,
                                     accum_out=slot(1))             # A3
                nc.scalar.activation(u2[:], u2[:], AF.Sqrt, 0.0, 4.0,
                                     accum_out=slot(2))             # A4
                nc.scalar.activation(pd, pd, AF.Square,
                                     accum_out=slot(3))             # A5
                nc.scalar.activation(mpc[:], mpc[:], AF.Square,
                                     accum_out=slot(4))             # A6
                nc.scalar.activation(pc_plane, pc_plane, AF.Square,
                                     accum_out=slot(5))             # A7

            nc.sync.dma_start(out_d[:], acc[:])

    nc.compile()
    return nc


_nc_cache = {}


def get_nc(F=F):
    if F not in _nc_cache:
        _nc_cache[F] = build_nc(F)
    return _nc_cache[F]


def make_in_maps(pred_tensor, target_boxes, obj_mask):
    pred = np.asarray(pred_tensor, dtype=np.float32).reshape(B, CELLS, 5)
    targ = np.asarray(target_boxes, dtype=np.float32).reshape(B, CELLS, 4)
    mask = np.asarray(obj_mask).reshape(B, CELLS)

    X = np.empty((B, 9, CELLS), dtype=np.float32)
    X[:, 0] = pred[:, :, 0]   # px
    X[:, 1] = targ[:, :, 0]   # tx
    X[:, 2] = pred[:, :, 1]   # py
    X[:, 3] = targ[:, :, 1]   # ty
    X[:, 4] = pred[:, :, 2]   # pw
    X[:, 5] = targ[:, :, 2]   # tw
    X[:, 6] = pred[:, :, 3]   # ph
    X[:, 7] = targ[:, :, 3]   # th
    X[:, 8] = pred[:, :, 4]   # pc
    M = (mask != 0).astype(np.uint8)

    X = X.reshape(N_CORES, PB, 9 * CELLS)
    M = np.ascontiguousarray(M.reshape(N_CORES, PB, CELLS))
    return [{"x": X[k], "m": M[k]} for k in range(N_CORES)]


def combine_accs(accs, nchunk=NCHUNK):
    """accs: list of per-core [PB, NG*nchunk] f32 partial sums."""
    a = np.asarray(accs, dtype=np.float64)
    a = a.reshape(len(accs), PB, NG, nchunk)
    S = a.sum(axis=(0, 1, 3))                   # [NG]
    A12, A3, A4, A5, A6, A7 = S
    loss_sum = 5.0 * (A12 + A3 - A4) + A5 + 0.5 * (A7 - A6)
    return np.float32(loss_sum / B)


def kernel(pred_tensor, target_boxes, obj_mask):
    nc = get_nc()
    in_maps = make_in_maps(pred_tensor, target_boxes, obj_mask)
    res = run_bass_kernel_spmd(nc, in_maps, core_ids=list(range(N_CORES)))
    accs = [res.results[k]["acc"] for k in range(N_CORES)]
    return combine_accs(accs)


if __name__ == "__main__":
    rng = np.random.default_rng(0)
    p = rng.random((B, 80, 80, 5), dtype=np.float32)
    t = rng.random((B, 80, 80, 4), dtype=np.float32)
    m = rng.integers(0, 2, size=(B, 80, 80)).astype(np.int32)
    print("loss:", kernel(p, t, m))
